# revision 8
# baseline (speedup 1.0000x reference)
"""FAGCN message-passing layer on 8 Trainium2 NeuronCores (Bass/Tile).

Strategy (v4: int8 table rows + PE identity-matmul segment reduction):
  - Nodes 1D-partitioned across 8 cores by dst (12544/core), degree-
    sorted into 98 windows of 128; window w partition p owns one dst.
  - Per-core node TABLE (4 int16-range tensors, 256B rows, host-staged):
    row = [q int8[64] (h per-row-max-quantized), scale*d_src f16, gs f16].
    gs = h @ W_src is computed ON DEVICE (DVE mult + tree reduce over a
    dense f16 copy of the table rows) and written into the 2-byte column.
  - Main loop: one raw dma_gather per (window-group, range) fetches 68B
    rows (the 7ns/desc DMA floor) at ~0.40-0.44 ns/edge; per-window DVE
    gd-add + one batched ACT tanh; th2 pair tile; ACT bulk int8->f16
    dequant; one DVE 2x multiply applies tanh*scale per slot.
  - Reduction on the TENSOR engine: per slot-column matmul with a
    constant 128x128 identity as stationary accumulates messages into
    per-window PSUM tiles (start/stop once per 2KB bank); ACT evacuates
    PSUM with the d_dst scale fused. z un-permuted on the host.
"""
import numpy as np

P = 128
D = 64
EL = 256          # table row stride bytes
CB = 68           # gathered content bytes: 64 q + 2 scale' + 2 gs
N_CORES = 8
NPC = 12544
NW = NPC // P     # 98
N_NODES_MAX = 100352
R2 = 102400       # total table rows
NRANGE = 4
RSTART = [0, 32768, 65536, 98304]
RCAP = [32767, 32767, 32767, 4095]   # last row of each range = zero row
ZROWR = [32767, 32767, 32767, 4095]  # in-range index of the zero row
GRP = 16          # windows per gather group
GRPS = [list(range(w0, min(w0 + GRP, NW))) for w0 in range(0, NW, GRP)]
# PSUM sections: 32 windows = 4 banks each; ring of 2 tiles = 8 banks
SECS = [GRPS[0:2], GRPS[2:4], GRPS[4:6], GRPS[6:]]
SW0 = [0, 32, 64, 96]                # first window of each section
SNW = [32, 32, 32, 2]                # windows per section
GS_CHUNK = 64                        # hdense j-columns per gs chunk


def _color_ranges(src_e, dl_e, npc):
    """Greedy balanced range coloring: assign each referenced src node a
    range 0..2 (overflow 3) minimizing per-dst edge imbalance."""
    order_e = np.argsort(src_e, kind="stable")
    ss = src_e[order_e]
    dd = dl_e[order_e]
    uniq, starts = np.unique(ss, return_index=True)
    ends = np.append(starts[1:], ss.size)
    refcnt = ends - starts
    proc = np.argsort(-refcnt, kind="stable")
    color = np.full(N_NODES_MAX, 3, np.int8)
    cnt = np.zeros((npc, 3), np.int32)
    fill = [0, 0, 0]
    for k in proc:
        s = uniq[k]
        dsts = dd[starts[k]:ends[k]]
        score = cnt[dsts, :].sum(axis=0)
        for g in np.argsort(score, kind="stable"):
            if fill[g] < RCAP[g]:
                break
        else:
            g = 3
        color[s] = g
        if g < 3:
            fill[g] += 1
            np.add.at(cnt, (dsts, g), 1)
    return color, uniq, refcnt


def _idx_layout(NTWG):
    """Idx column offsets: blocks ordered (half, g, grp, w) so each
    (half, g, grp) gather's indices are one contiguous block."""
    icol_off = np.zeros((NW, NRANGE), np.int64)
    c = 0
    for sec in SECS:
        for g in range(NRANGE):
            for ws in sec:
                for w in ws:
                    icol_off[w, g] = c
                    c += int(NTWG[w, g]) * 8
    return icol_off, c


def _mm_stream(NTWG):
    """Per-bank first/last matmul (g, w, t) in emission order."""
    first = {}
    last = {}
    for sec in SECS:
        for g in range(NRANGE):
            for ws in sec:
                for w in ws:
                    for t0 in range(0, int(NTWG[w, g]), 8):
                        b = w // 8
                        if b not in first:
                            first[b] = (g, w, t0)
                        last[b] = (g, w, t0)
    return first, last


def _host_prep(h, d, gate_W, gate_b, edge_src, edge_dst):
    """Shard + layout preparation (data movement, permutation, int8
    transport quantization). All gate-projection FLOPs stay on device."""
    N = h.shape[0]
    h32 = np.asarray(h, dtype=np.float32)
    h_pad = np.zeros((N_NODES_MAX, D), dtype=np.float32)
    h_pad[:N] = h32
    d_pad = np.zeros((N_NODES_MAX,), dtype=np.float32)
    d_pad[:N] = np.asarray(d, dtype=np.float32)

    # int8 transport quantization of h (per-row scale)
    scale = np.abs(h_pad).max(axis=1) / 127.0
    safe = np.where(scale > 0, scale, 1.0)
    q_all = np.clip(np.round(h_pad / safe[:, None]), -127, 127).astype(np.int8)
    h16_all = h_pad.astype(np.float16)

    WSRC = np.tile(np.asarray(gate_W[0, D:2 * D], np.float16), (P, 1))
    WDST = np.tile(np.asarray(gate_W[0, 0:D], np.float16), (P, 1))
    BREP = np.full((P, 1), float(np.asarray(gate_b).reshape(-1)[0]), np.float32)
    IDENT = np.eye(P, dtype=np.float16)

    order = np.argsort(edge_dst, kind="stable")
    sd = np.asarray(edge_dst)[order].astype(np.int64)
    ss = np.asarray(edge_src)[order].astype(np.int64)
    bounds = np.searchsorted(sd, np.arange(N_CORES + 1) * NPC)

    cores = []
    for c in range(N_CORES):
        lo, hi = int(bounds[c]), int(bounds[c + 1])
        dl = sd[lo:hi] - c * NPC
        src = ss[lo:hi]

        color, uniq, refcnt = _color_ranges(src, dl, NPC)

        # table row assignment: per range, referenced srcs by refcount desc
        tau = np.full(N_NODES_MAX, -1, np.int64)
        g_all = np.full(N_NODES_MAX, -1, np.int8)
        g_all[uniq] = color[uniq]
        used = np.zeros(NRANGE, np.int64)
        rc_full = np.zeros(N_NODES_MAX, np.int64)
        rc_full[uniq] = refcnt
        for g in range(NRANGE):
            nodes_g = uniq[color[uniq] == g]
            nodes_g = nodes_g[np.argsort(-rc_full[nodes_g], kind="stable")]
            assert nodes_g.size <= RCAP[g], (g, nodes_g.size)
            tau[nodes_g] = RSTART[g] + np.arange(nodes_g.size)
            used[g] = nodes_g.size
        unref = np.where(g_all < 0)[0]
        pos = 0
        for g in range(NRANGE):
            free = RCAP[g] - used[g]
            take = min(free, unref.size - pos)
            if take > 0:
                tau[unref[pos:pos + take]] = RSTART[g] + used[g] + np.arange(take)
                used[g] += take
                pos += take
        assert pos == unref.size

        # per-edge range + rank within (dst, range)
        ge = g_all[src].astype(np.int64)
        key = dl * NRANGE + ge
        c_g = np.bincount(key, minlength=NPC * NRANGE).reshape(NPC, NRANGE)
        deg = c_g.sum(axis=1)
        kstart = np.zeros(NPC * NRANGE, np.int64)
        kstart[1:] = np.cumsum(np.bincount(key, minlength=NPC * NRANGE))[:-1]
        ord2 = np.argsort(key, kind="stable")
        trank = np.empty(src.size, np.int64)
        trank[ord2] = np.arange(src.size) - kstart[key[ord2]]

        # window packing: group dsts by worst per-range count
        perm = np.argsort(-(c_g[:, :3].max(axis=1) * 64 + deg), kind="stable")
        rankof = np.empty(NPC, np.int64)
        rankof[perm] = np.arange(NPC)

        ntwg_c = c_g[perm].reshape(NW, P, NRANGE).max(axis=1)  # [NW, 4]
        cores.append(dict(
            dl=dl, src=src, tau=tau, ge=ge, trank=trank, perm=perm,
            rankof=rankof, ntwg=ntwg_c,
        ))

    NTWG = np.maximum.reduce([cc["ntwg"] for cc in cores])     # [NW, 4]
    # every PSUM bank needs >=1 matmul so its start=True zeroing fires
    for w in range(0, NW, 8):
        NTWG[w, 0] = max(NTWG[w, 0], 1)
    key = tuple(int(x) for x in NTWG.reshape(-1))
    icol_off, TOTI = _idx_layout(NTWG)

    in_maps = []
    for c in range(N_CORES):
        cc = cores[c]
        tau, ge, trank, perm, rankof = (cc["tau"], cc["ge"], cc["trank"],
                                        cc["perm"], cc["rankof"])
        dl, src = cc["dl"], cc["src"]

        r = rankof[dl]
        p_arr = r % P
        w_arr = r // P

        # idx stream: per (w,g) block, one int16 per slot at
        # [sel%16, off+sel//16], replicated x8 across partition groups
        idxw = np.zeros((16, TOTI), np.int16)
        for w in range(NW):
            for g in range(NRANGE):
                nt = int(NTWG[w, g])
                if nt == 0:
                    continue
                o = int(icol_off[w, g])
                idxw[:, o:o + nt * 8] = ZROWR[g]
        sel_i = trank * P + p_arr
        col16 = sel_i // 16
        row16 = sel_i % 16
        val = (tau[src] - np.asarray(RSTART, np.int64)[ge]).astype(np.int64)
        dest_col = icol_off[w_arr, ge] + col16
        idxw[row16, dest_col] = val.astype(np.int16)
        IDXW = np.tile(idxw, (8, 1))

        # node id for each table row
        node_of_row = np.full(R2, -1, np.int64)
        node_of_row[tau[tau >= 0]] = np.where(tau >= 0)[0]
        m = node_of_row >= 0
        rows_n = np.where(m, node_of_row, 0)

        # int8 table per range: [q 0:64 | scale*d_src f16 64:66 | gs 66:68]
        tabs = {}
        hdense = np.zeros((P, R2 // P, D), np.float16)
        for g in range(NRANGE):
            nr = RCAP[g] + 1
            rs = slice(RSTART[g], RSTART[g] + nr)
            tab = np.zeros((nr, EL), np.int8)
            mg = m[rs]
            tab[mg, 0:D] = q_all[rows_n[rs][mg]]
            scp = np.zeros(nr, np.float16)
            scp[mg] = (scale[rows_n[rs][mg]]
                       * d_pad[rows_n[rs][mg]]).astype(np.float16)
            scp[ZROWR[g]] = 0.0
            tab[ZROWR[g], :] = 0
            tab[:, D:D + 2] = scp.view(np.int8).reshape(nr, 2)
            tabs[f"tab{g}"] = tab
        # dense f16 rows for on-device gs compute: hdense[p, j] = h16[row j*128+p]
        hd = np.zeros((R2, D), np.float16)
        hd[m] = h16_all[node_of_row[m]]
        hdense = np.ascontiguousarray(
            hd.reshape(R2 // P, P, D).transpose(1, 0, 2))

        # local dst features (window-ordered) for gd; d_dst column
        nodes_loc = perm + c * NPC
        hloc = np.ascontiguousarray(
            h16_all[nodes_loc].reshape(NW, P, D).transpose(1, 0, 2))
        dcol = np.ascontiguousarray(
            d_pad[nodes_loc].reshape(NW, P).T).astype(np.float32)

        in_maps.append({
            **tabs, "hdense": hdense, "hloc": hloc, "dcol": dcol,
            "wsrc": WSRC, "wdst": WDST, "brep": BREP, "ident": IDENT,
            "idxw": np.ascontiguousarray(IDXW),
            "_perm": perm,
        })
    return in_maps, key


def _raw_gather(eng, mybir, out_ap, in_ap, idxs_ap, num_idxs, elem_bytes):
    """dma_gather with elem_size < 256B (content-only fetch); row stride
    encoded in 256B units. Semantics verified on hardware (v3) and in the
    interpreter (micro tests)."""
    _in_ap = eng.lower_ap_dma(in_ap, for_custom_bir_dma=True)
    _idxs_ap = eng.lower_ap(idxs_ap)
    _out_ap = eng.lower_ap(out_ap)
    return eng.add_instruction(
        mybir.InstDMAGatherAnt(
            name=eng.bass.get_next_instruction_name(),
            ins=[*_in_ap, _idxs_ap,
                 eng.lower_val_access(eng.to_reg(num_idxs))],
            outs=[_out_ap],
            transpose=False,
            num_idxs=num_idxs,
            elem_size=elem_bytes,
            stride_bytes_256=EL // 256,
            gen_mode=0,
            single_packet=False,
            queue_num=0,
            sbuf_tokens_per_rank=0,
            sbuf_free_dim_per_rank=0,
            sbuf_free_dim_pad_per_rank=0,
            sbuf_byte_offset=0,
        ))


def _build_program(key):
    import concourse.bacc as bacc
    import concourse.tile as tile
    from concourse import bass, mybir

    NTWG = np.asarray(key, np.int64).reshape(NW, NRANGE)
    icol_off, TOTI = _idx_layout(NTWG)
    first_mm, last_mm = _mm_stream(NTWG)
    S2MAX = int(max(sum(int(NTWG[w, g]) for w in ws)
                    for ws in GRPS for g in range(NRANGE)))
    S2MAX = max(S2MAX, 1)

    f32, f16 = mybir.dt.float32, mybir.dt.float16
    i16, i8 = mybir.dt.int16, mybir.dt.int8

    nc = bacc.Bacc("TRN2", target_bir_lowering=False, debug=False,
                   num_devices=N_CORES)
    PE = nc.engines[mybir.EngineType.PE]
    tab_d = [nc.dram_tensor(f"tab{g}", [RCAP[g] + 1, EL], i8,
                            kind="ExternalInput") for g in range(NRANGE)]
    hdense_d = nc.dram_tensor("hdense", [P, R2 // P, D], f16,
                              kind="ExternalInput")
    hloc_d = nc.dram_tensor("hloc", [P, NW, D], f16, kind="ExternalInput")
    dcol_d = nc.dram_tensor("dcol", [P, NW], f32, kind="ExternalInput")
    wsrc_d = nc.dram_tensor("wsrc", [P, D], f16, kind="ExternalInput")
    wdst_d = nc.dram_tensor("wdst", [P, D], f16, kind="ExternalInput")
    brep_d = nc.dram_tensor("brep", [P, 1], f32, kind="ExternalInput")
    ident_d = nc.dram_tensor("ident", [P, P], f16, kind="ExternalInput")
    idxw_d = nc.dram_tensor("idxw", [P, TOTI], i16, kind="ExternalInput")
    z_d = nc.dram_tensor("z", [P, NW, D], f16, kind="ExternalOutput")

    with tile.TileContext(nc) as tc:
        with tc.tile_pool(name="const", bufs=1) as cp, \
             tc.tile_pool(name="gsb", bufs=2) as gp, \
             tc.tile_pool(name="mainb", bufs=2) as mpb, \
             tc.tile_pool(name="mains", bufs=3) as mps, \
             tc.psum_pool(name="pp", bufs=2) as pp:
            dcol_t = cp.tile([P, NW], f32)
            nc.sync.dma_start(out=dcol_t[:], in_=dcol_d[:, :])
            wsrc_t = cp.tile([P, D], f16)
            nc.sync.dma_start(out=wsrc_t[:], in_=wsrc_d[:, :])
            wdst_t = cp.tile([P, D], f16)
            nc.sync.dma_start(out=wdst_t[:], in_=wdst_d[:, :])
            brep_t = cp.tile([P, 1], f32)
            nc.sync.dma_start(out=brep_t[:], in_=brep_d[:, :])
            ident_t = cp.tile([P, P], f16)
            nc.sync.dma_start(out=ident_t[:], in_=ident_d[:, :])

            # ---- gd for local (window-permuted) dst nodes ----
            hloc_t = cp.tile([P, NW, D], f16)
            nc.sync.dma_start(out=hloc_t[:], in_=hloc_d[:, :, :])
            nc.vector.tensor_tensor(
                out=hloc_t[:], in0=hloc_t[:],
                in1=wdst_t[:].rearrange("p (a e) -> p a e",
                                        a=1).to_broadcast([P, NW, D]),
                op=mybir.AluOpType.mult)
            width = D
            while width > 2:
                half = width // 2
                nc.vector.tensor_tensor(
                    out=hloc_t[:, :, 0:half], in0=hloc_t[:, :, 0:half],
                    in1=hloc_t[:, :, half:width], op=mybir.AluOpType.add)
                width = half
            gdw_t = cp.tile([P, NW], f32)
            nc.vector.tensor_tensor(
                out=gdw_t[:], in0=hloc_t[:, :, 0],
                in1=hloc_t[:, :, 1], op=mybir.AluOpType.add)
            nc.vector.tensor_scalar(
                out=gdw_t[:], in0=gdw_t[:], scalar1=brep_t[:, 0:1],
                scalar2=None, op0=mybir.AluOpType.add)

            # ---- gs per range: dense f16 rows -> h @ W_src -> table col ----
            def emit_gs_range(g):
                nj = (RCAP[g] + 1) // P          # j-columns in this range
                j0 = RSTART[g] // P
                gs_g = gp.tile([P, 256], f16, tag="gsg")
                for s in range(0, nj, GS_CHUNK):
                    w_ = min(GS_CHUNK, nj - s)
                    hd8 = gp.tile([P, GS_CHUNK, D], f16, tag="hd8")
                    nc.sync.dma_start(out=hd8[:, 0:w_, :],
                                      in_=hdense_d[:, j0 + s:j0 + s + w_, :])
                    nc.vector.tensor_tensor(
                        out=hd8[:, 0:w_, :], in0=hd8[:, 0:w_, :],
                        in1=wsrc_t[:].rearrange("p (a e) -> p a e",
                                                a=1).to_broadcast([P, w_, D]),
                        op=mybir.AluOpType.mult)
                    width = D
                    while width > 1:
                        half = width // 2
                        nc.vector.tensor_tensor(
                            out=hd8[:, 0:w_, 0:half], in0=hd8[:, 0:w_, 0:half],
                            in1=hd8[:, 0:w_, half:width],
                            op=mybir.AluOpType.add)
                        width = half
                    nc.vector.tensor_copy(out=gs_g[:, s:s + w_],
                                          in_=hd8[:, 0:w_, 0])
                view = tab_d[g][:, 66:68].bitcast(f16).rearrange(
                    "(j p) one -> p j one", p=P)
                nc.sync.dma_start(out=view, in_=gs_g[:, 0:nj].rearrange(
                    "p (j one) -> p j one", one=1))

            emit_gs_range(0)

            # ---- gdexp: per-slot gd values in block-concatenated layout ----
            boff = {}
            tot = 0
            for si, sec in enumerate(SECS):
                for g in range(NRANGE):
                    for gi, ws in enumerate(sec):
                        boff[(si, g, gi)] = tot
                        tot += sum(int(NTWG[w, g]) for w in ws)
            gdexp_t = cp.tile([P, max(tot, 1)], f16)
            for si, sec in enumerate(SECS):
                for g in range(NRANGE):
                    for gi, ws in enumerate(sec):
                        o = boff[(si, g, gi)]
                        for w in ws:
                            nt = int(NTWG[w, g])
                            if nt == 0:
                                continue
                            nc.vector.tensor_copy(
                                out=gdexp_t[:, o:o + nt],
                                in_=gdw_t[:, w:w + 1].to_broadcast([P, nt]))
                            o += nt

            # ---- main loop: sections x ranges x groups ----
            pending_gs = [g for g in (1, 2, 3)
                          if int(NTWG[:, g].sum()) > 0]

            for si, sec in enumerate(SECS):
                psec = pp.tile([P, 32, D], f32, tag="ps")
                for g in range(NRANGE):
                    # overlap later-range gs builds with earlier passes
                    if pending_gs:
                        emit_gs_range(pending_gs.pop(0))
                    for ws in sec:
                        S2 = sum(int(NTWG[w, g]) for w in ws)
                        if S2 == 0:
                            continue
                        toff = {}
                        t_ = 0
                        for w in ws:
                            toff[w] = t_
                            t_ += int(NTWG[w, g])
                        blk = int(icol_off[ws[0], g])

                        ga = mpb.tile([P, S2MAX, CB], i8, tag="ga")
                        th = mps.tile([P, S2MAX], f16, tag="th")
                        th2 = mps.tile([P, S2MAX, 2], f16, tag="th2")
                        msgf = mpb.tile([P, S2MAX, D], f16, tag="msgf")
                        idx_t = mps.tile([P, S2MAX * 8], i16, tag="idx")
                        nc.sync.dma_start(
                            out=idx_t[:, 0:S2 * 8],
                            in_=idxw_d[:, blk:blk + S2 * 8])
                        for c0 in range(0, S2, 64):
                            cn = min(64, S2 - c0)
                            _raw_gather(nc.gpsimd, mybir,
                                        ga[:, c0:c0 + cn, :],
                                        tab_d[g][:, 0:CB],
                                        idx_t[:, c0 * 8:(c0 + cn) * 8],
                                        cn * P, CB)
                        gs_ap = ga[:, 0:S2, 66:68].bitcast(f16)
                        sc_ap = ga[:, 0:S2, 64:66].bitcast(f16)
                        bo = boff[(si, g, sec.index(ws))]
                        nc.vector.tensor_tensor(
                            out=th[:, 0:S2],
                            in0=gs_ap[:, 0:S2, 0],
                            in1=gdexp_t[:, bo:bo + S2],
                            op=mybir.AluOpType.add)
                        nc.scalar.activation(
                            out=th[:, 0:S2], in_=th[:, 0:S2],
                            func=mybir.ActivationFunctionType.Tanh)
                        nc.vector.scalar_tensor_tensor(
                            out=th2[:, 0:S2, :],
                            in0=th[:, 0:S2].rearrange(
                                "p (s a) -> p s a", a=1).to_broadcast([P, S2, 2]),
                            scalar=1.0,
                            in1=sc_ap.to_broadcast([P, S2, 2]),
                            op0=mybir.AluOpType.mult, op1=mybir.AluOpType.mult)
                        nc.scalar.activation(
                            out=msgf[:, 0:S2, :], in_=ga[:, 0:S2, 0:D],
                            func=mybir.ActivationFunctionType.Copy)
                        nc.vector.tensor_tensor(
                            out=msgf[:, 0:S2, :].rearrange(
                                "p s (c two) -> p s c two", two=2),
                            in0=msgf[:, 0:S2, :].rearrange(
                                "p s (c two) -> p s c two", two=2),
                            in1=th2[:, 0:S2, :].rearrange(
                                "p s (a two) -> p s a two", a=1
                            ).to_broadcast([P, S2, D // 2, 2]),
                            op=mybir.AluOpType.mult)
                        for w in ws:
                            nt = int(NTWG[w, g])
                            if nt == 0:
                                continue
                            o = toff[w]
                            b = w // 8
                            for t0 in range(0, nt, 8):
                                k = min(8, nt - t0)
                                PE.matmul(
                                    out=psec[:, w - SW0[si], :].rearrange(
                                        "p (a d) -> p a d", a=1
                                    ).to_broadcast([P, k, D]),
                                    lhsT=ident_t[:],
                                    rhs=msgf[:, o + t0:o + t0 + k, :].rearrange(
                                        "p k d -> p (k d)"),
                                    start=(first_mm[b] == (g, w, t0)),
                                    stop=(last_mm[b] == (g, w, t0)))
                # evacuate this section's PSUM with d_dst fused
                zo = mps.tile([P, 32, D], f16, tag="zo")
                for w in range(SW0[si], SW0[si] + SNW[si]):
                    nc.scalar.activation(
                        out=zo[:, w - SW0[si], :],
                        in_=psec[:, w - SW0[si], :],
                        func=mybir.ActivationFunctionType.Copy,
                        scale=dcol_t[:, w:w + 1])
                nc.sync.dma_start(
                    out=z_d[:, SW0[si]:SW0[si] + SNW[si], :],
                    in_=zo[:, 0:SNW[si], :])

    nc.compile()
    return nc


_CACHE = {}


def kernel(h, d, gate_W, gate_b, edge_src, edge_dst):
    from concourse.bass_utils import run_bass_kernel_spmd

    N = h.shape[0]
    in_maps, key = _host_prep(h, d, gate_W, gate_b, edge_src, edge_dst)
    if key not in _CACHE:
        _CACHE[key] = _build_program(key)
    nc = _CACHE[key]
    perms = [m.pop("_perm") for m in in_maps]
    res = run_bass_kernel_spmd(nc, in_maps, core_ids=list(range(N_CORES)))
    z = np.empty((N_CORES * NPC, D), np.float32)
    for c in range(N_CORES):
        zc = res.results[c]["z"].astype(np.float32)    # [128, NW, 64]
        zperm = zc.transpose(1, 0, 2).reshape(NPC, D)  # rank-major
        z[perms[c] + c * NPC] = zperm
    return np.ascontiguousarray(z[:N]).astype(np.float32)
